# revision 8
# baseline (speedup 1.0000x reference)
"""AnomalyAttention TRN2 kernel: 8-core SPMD (2 batch-groups x 4 n-groups).

Module math (B=16, N=1024, C=D=256):
  Q,K,V = x@Wq, x@Wk, x@Wv ; sigma = x@Wsig
  P = rownorm(|i-j| + |sigma|*eps)
  S = softmax over BATCH dim of (Q K^T / 16)
  Z = S @ V
Returns (Z, P, S) like the reference.

Sharding: core c handles batches bh*8..bh*8+8 (bh=c%2) and rows
ng*256..ng*256+256 (ng=c//2). The batch-softmax denominator needs a
2-rank AllReduce between pair (2g, 2g+1).

Device computes scores in TRANSPOSED orientation (m on partitions) so
S^T feeds the Z matmul directly; the host transposes S back. Matmuls
run in float32r (tf32-like, full PE speed). K is never materialized:
scoresT = x @ (Wq Wk^T)^T_slice @ x_cols via Mt = Wq@Wk^T on device.
"""
import sys
sys.path.insert(0, '/opt/trn_rl_repo')
import numpy as np

B, N, C, D = 16, 1024, 256, 256
N_CORES = 8
B_LOC = 8      # batches per core
N_LOC = 256    # n rows per core
NT = N_LOC // 128   # 2 n-tiles
MT = N // 128       # 8 m-tiles
KC = C // 128       # 2 contraction tiles

_cached = {}


def _build():
    import concourse.bass as bass
    import concourse.bacc as bacc
    import concourse.mybir as mybir
    from concourse import tile

    f32 = mybir.dt.float32
    f32r = mybir.dt.float32r
    AF = mybir.ActivationFunctionType
    ALU = mybir.AluOpType

    nc = bacc.Bacc("TRN2", target_bir_lowering=False, debug=False,
                   num_devices=N_CORES)

    xT = nc.dram_tensor("xT", [B_LOC, C, N], f32r, kind="ExternalInput")
    xq = nc.dram_tensor("xq", [B_LOC, C, N_LOC], f32r, kind="ExternalInput")
    eps = nc.dram_tensor("eps", [B_LOC, N_LOC, N], f32, kind="ExternalInput")
    p_in = nc.dram_tensor("p", [N_LOC, N], f32, kind="ExternalInput")
    wqT = nc.dram_tensor("wqT", [D, C], f32r, kind="ExternalInput")
    wkT = nc.dram_tensor("wkT", [D, C], f32r, kind="ExternalInput")
    wv = nc.dram_tensor("wv", [C, D], f32r, kind="ExternalInput")
    wsig = nc.dram_tensor("wsig", [C, 1], f32r, kind="ExternalInput")

    st_out = nc.dram_tensor("st_out", [B_LOC, N, N_LOC], f32,
                            kind="ExternalOutput")
    p_out = nc.dram_tensor("p_out", [B_LOC, N_LOC, N], f32,
                           kind="ExternalOutput")
    z_out = nc.dram_tensor("z_out", [B_LOC, N_LOC, D], f32,
                           kind="ExternalOutput")

    with tile.TileContext(nc) as tc:
        with tc.tile_pool(name="consts", bufs=1) as consts, \
             tc.tile_pool(name="xt", bufs=2) as xtp, \
             tc.tile_pool(name="xqp", bufs=2) as xqp, \
             tc.tile_pool(name="gp", bufs=2) as gp, \
             tc.tile_pool(name="big", bufs=1) as big, \
             tc.tile_pool(name="epsp", bufs=2) as epsp, \
             tc.tile_pool(name="dram", bufs=1, space="DRAM") as dram:

            # ---- constants ----
            wqT_sb = consts.tile([128, 2 * C], f32r)   # d-tile k at k*C
            wkT_sb = consts.tile([128, 2 * C], f32r)
            wv_sb = consts.tile([128, 2 * D], f32r)    # c-tile k at k*D
            wsig_sb = consts.tile([128, KC], f32r)
            p_sb = consts.tile([128, NT * N], f32)     # n-tile at nt*N
            mt_sb = consts.tile([128, 2 * C], f32r)    # Mt: c'-tile at k*C
            sig_sb = consts.tile([128, B_LOC * NT], f32)
            rs_sb = consts.tile([128, B_LOC * NT], f32)
            rinv_sb = consts.tile([128, B_LOC * NT], f32)

            for k in range(KC):
                nc.sync.dma_start(wqT_sb[:, k * C:(k + 1) * C],
                                  wqT.ap()[k * 128:(k + 1) * 128, :])
                nc.sync.dma_start(wkT_sb[:, k * C:(k + 1) * C],
                                  wkT.ap()[k * 128:(k + 1) * 128, :])
                nc.sync.dma_start(wv_sb[:, k * D:(k + 1) * D],
                                  wv.ap()[k * 128:(k + 1) * 128, :])
                nc.sync.dma_start(wsig_sb[:, k:k + 1],
                                  wsig.ap()[k * 128:(k + 1) * 128, :])
            for nt in range(NT):
                nc.sync.dma_start(p_sb[:, nt * N:(nt + 1) * N],
                                  p_in.ap()[nt * 128:(nt + 1) * 128, :])

            # ---- big persistent tiles ----
            expT = big.tile([128, MT * B_LOC * 256], f32)  # 8MB: mt-major, b at b*256
            d_loc = big.tile([128, MT * 256], f32)         # 1MB; reused as R after AR
            v_all = big.tile([128, B_LOC * MT * 256], f32r)  # 8MB: (b*8+mt)*256

            ar_in = dram.tile([128, MT * 256], f32)
            ar_out = dram.tile([128, MT * 256], f32)

            with tc.tile_pool(name="psab", bufs=4, space="PSUM") as psab:
                # Mt = Wq @ Wk^T  (lhsT = wqT slices, rhs = wkT)
                for cpt in range(KC):
                    ps = psab.tile([128, 512], f32, tag="ps")
                    for kd in range(KC):
                        nc.tensor.matmul(
                            ps[:, 0:C],
                            wqT_sb[:, kd * C + cpt * 128: kd * C + cpt * 128 + 128],
                            wkT_sb[:, kd * C:(kd + 1) * C],
                            start=(kd == 0), stop=(kd == KC - 1))
                    nc.vector.tensor_copy(mt_sb[:, cpt * C:(cpt + 1) * C],
                                          ps[:, 0:C])

                for b in range(B_LOC):
                    # ---- loads ----
                    xt_t = xtp.tile([128, KC * N], f32r, tag="xt")
                    for k in range(KC):
                        nc.sync.dma_start(xt_t[:, k * N:(k + 1) * N],
                                          xT.ap()[b, k * 128:(k + 1) * 128, :])
                    xq_t = xqp.tile([128, KC * N_LOC], f32r, tag="xq")
                    for k in range(KC):
                        nc.sync.dma_start(xq_t[:, k * N_LOC:(k + 1) * N_LOC],
                                          xq.ap()[b, k * 128:(k + 1) * 128, :])

                    # ---- G[b] = Mt^T-style: out [c-tile, n] ----
                    g_t = gp.tile([128, KC * N_LOC], f32r, tag="g")
                    for ct in range(KC):
                        ps = psab.tile([128, 512], f32, tag="ps")
                        for kcp in range(KC):
                            nc.tensor.matmul(
                                ps[:, 0:N_LOC],
                                mt_sb[:, kcp * C + ct * 128: kcp * C + ct * 128 + 128],
                                xq_t[:, kcp * N_LOC:(kcp + 1) * N_LOC],
                                start=(kcp == 0), stop=(kcp == KC - 1))
                        nc.vector.tensor_copy(
                            g_t[:, ct * N_LOC:(ct + 1) * N_LOC],
                            ps[:, 0:N_LOC])

                    # ---- sigma[b] ----
                    for nt in range(NT):
                        ps = psab.tile([128, 512], f32, tag="ps")
                        for k in range(KC):
                            nc.tensor.matmul(
                                ps[:, 0:1],
                                xq_t[:, k * N_LOC + nt * 128:
                                     k * N_LOC + nt * 128 + 128].bitcast(f32),
                                wsig_sb[:, k:k + 1].bitcast(f32),
                                start=(k == 0), stop=(k == KC - 1))
                        slot = b * NT + nt
                        nc.scalar.activation(sig_sb[:, slot:slot + 1],
                                             ps[:, 0:1], AF.Abs)

                    # ---- scoresT + exp ----
                    for mt in range(MT):
                        ps = psab.tile([128, 512], f32, tag="ps")
                        for k in range(KC):
                            nc.tensor.matmul(
                                ps[:, 0:N_LOC],
                                xt_t[:, k * N + mt * 128: k * N + mt * 128 + 128],
                                g_t[:, k * N_LOC:(k + 1) * N_LOC],
                                start=(k == 0), stop=(k == KC - 1))
                        off = mt * B_LOC * 256 + b * 256
                        nc.scalar.activation(expT[:, off:off + 256],
                                             ps[:, 0:N_LOC], AF.Exp,
                                             scale=1.0 / 16.0)

                    # ---- V[b] ----
                    for mt in range(MT):
                        ps = psab.tile([128, 512], f32, tag="ps")
                        for k in range(KC):
                            nc.tensor.matmul(
                                ps[:, 0:D],
                                xt_t[:, k * N + mt * 128: k * N + mt * 128 + 128],
                                wv_sb[:, k * D:(k + 1) * D],
                                start=(k == 0), stop=(k == KC - 1))
                        voff = (b * MT + mt) * 256
                        nc.vector.tensor_copy(v_all[:, voff:voff + 256],
                                              ps[:, 0:D])

                    # ---- incremental D sums after each odd b ----
                    if b % 2 == 1:
                        for mt in range(MT):
                            base = mt * B_LOC * 256
                            e0 = expT[:, base + (b - 1) * 256: base + b * 256]
                            e1 = expT[:, base + b * 256: base + (b + 1) * 256]
                            dsl = d_loc[:, mt * 256:(mt + 1) * 256]
                            if b == 1:
                                nc.vector.tensor_add(dsl, e0, e1)
                            else:
                                nc.vector.tensor_add(dsl, dsl, e0)
                                nc.vector.tensor_add(dsl, dsl, e1)

                    # ---- P phase for this b ----
                    for nt in range(NT):
                        slot = b * NT + nt
                        ep = epsp.tile([128, N], f32, tag="eps")
                        nc.sync.dma_start(ep[:],
                                          eps.ap()[b, nt * 128:(nt + 1) * 128, :])
                        nc.vector.scalar_tensor_tensor(
                            ep[:], ep[:], sig_sb[:, slot:slot + 1],
                            p_sb[:, nt * N:(nt + 1) * N],
                            ALU.mult, ALU.add,
                            accum_out=rs_sb[:, slot:slot + 1])
                        nc.vector.reciprocal(rinv_sb[:, slot:slot + 1],
                                             rs_sb[:, slot:slot + 1])
                        nc.scalar.activation(ep[:], ep[:], AF.Copy,
                                             scale=rinv_sb[:, slot:slot + 1])
                        nc.sync.dma_start(
                            p_out.ap()[b, nt * 128:(nt + 1) * 128, :], ep[:])

                # ---- AllReduce of d_loc between pair ----
                nc.sync.dma_start(ar_in[:], d_loc[:])
                nc.gpsimd.collective_compute(
                    "AllReduce", ALU.add,
                    replica_groups=[[0, 1], [2, 3], [4, 5], [6, 7]],
                    ins=[ar_in.opt()], outs=[ar_out.opt()])
                nc.sync.dma_start(d_loc[:], ar_out[:])
                nc.vector.reciprocal_approx_fast(d_loc[:], d_loc[:])

            # ---- phase D: S^T, S out, Z ----
            with tc.tile_pool(name="psz", bufs=1, space="PSUM") as psz, \
                 tc.tile_pool(name="zsbp", bufs=2) as zsbp, \
                 tc.tile_pool(name="stp", bufs=2) as stp:
                zps = [psz.tile([128, 512], f32, tag=f"z{i}",
                                name=f"zp{i}") for i in range(B_LOC)]
                for mt in range(MT):
                    sl = expT[:, mt * B_LOC * 256:(mt + 1) * B_LOC * 256]
                    src = sl.rearrange("p (b n) -> p b n", b=B_LOC)
                    st_t = stp.tile([128, B_LOC * 256], f32r, tag="st")
                    dst = st_t.rearrange("p (b n) -> p b n", b=B_LOC)
                    rb = (d_loc[:, mt * 256:(mt + 1) * 256]
                          .rearrange("p (o n) -> p o n", o=1)
                          .broadcast_to([128, B_LOC, 256]))
                    nc.vector.tensor_mul(dst, src, rb)
                    # write S^T slab: [128(m), b, n] -> st_out[b, m, n]
                    nc.sync.dma_start(
                        st_out.ap()[:, mt * 128:(mt + 1) * 128, :]
                        .rearrange("b m n -> m b n"),
                        st_t.bitcast(f32))
                    # Z accumulation
                    for b in range(B_LOC):
                        for nt in range(NT):
                            lhs = st_t[:, b * 256 + nt * 128:
                                       b * 256 + nt * 128 + 128]
                            voff = (b * MT + mt) * 256
                            nc.tensor.matmul(
                                zps[b][:, nt * 256:(nt + 1) * 256],
                                lhs, v_all[:, voff:voff + 256],
                                start=(mt == 0 and nt == 0),
                                stop=(mt == MT - 1 and nt == NT - 1),
                                skip_group_check=True)
                for b in range(B_LOC):
                    zsb = zsbp.tile([128, 512], f32, tag="zsb")
                    nc.scalar.activation(zsb[:], zps[b][:], AF.Copy)
                    for nt in range(NT):
                        nc.sync.dma_start(
                            z_out.ap()[b, nt * 128:(nt + 1) * 128, :],
                            zsb[:, nt * 256:(nt + 1) * 256])
    nc.compile()
    return nc


def _get_nc():
    if "nc" not in _cached:
        _cached["nc"] = _build()
    return _cached["nc"]


def kernel(x, W_q, W_k, W_v, W_sigma, eps):
    from concourse.bass_utils import run_bass_kernel_spmd

    x = np.ascontiguousarray(x, dtype=np.float32)
    eps = np.ascontiguousarray(eps, dtype=np.float32)
    W_q = np.ascontiguousarray(W_q, dtype=np.float32)
    W_k = np.ascontiguousarray(W_k, dtype=np.float32)
    W_v = np.ascontiguousarray(W_v, dtype=np.float32)
    W_sigma = np.ascontiguousarray(W_sigma, dtype=np.float32)

    xT_f = np.ascontiguousarray(x.transpose(0, 2, 1))  # [B, C, N]
    idx = np.arange(N, dtype=np.float32)
    p_full = np.abs(idx[:, None] - idx[None, :])       # [N, N]
    wqT = np.ascontiguousarray(W_q.T)
    wkT = np.ascontiguousarray(W_k.T)

    in_maps = []
    for c in range(N_CORES):
        bh, ng = c % 2, c // 2
        bs, ns = bh * B_LOC, ng * N_LOC
        in_maps.append({
            "xT": xT_f[bs:bs + B_LOC],
            "xq": np.ascontiguousarray(xT_f[bs:bs + B_LOC, :, ns:ns + N_LOC]),
            "eps": np.ascontiguousarray(eps[bs:bs + B_LOC, ns:ns + N_LOC, :]),
            "p": np.ascontiguousarray(p_full[ns:ns + N_LOC, :]),
            "wqT": wqT, "wkT": wkT, "wv": W_v, "wsig": W_sigma,
        })

    nc = _get_nc()
    res = run_bass_kernel_spmd(nc, in_maps, core_ids=list(range(N_CORES)))
    _cached["last_res"] = res

    Z = np.empty((B, N, D), dtype=np.float32)
    P = np.empty((B, N, N), dtype=np.float32)
    S = np.empty((B, N, N), dtype=np.float32)
    for c in range(N_CORES):
        bh, ng = c % 2, c // 2
        bs, ns = bh * B_LOC, ng * N_LOC
        r = res.results[c]
        Z[bs:bs + B_LOC, ns:ns + N_LOC, :] = r["z_out"]
        P[bs:bs + B_LOC, ns:ns + N_LOC, :] = r["p_out"]
        S[bs:bs + B_LOC, ns:ns + N_LOC, :] = r["st_out"].transpose(0, 2, 1)
    return Z, P, S


# revision 10
# speedup vs baseline: 1.0471x; 1.0471x over previous
"""AnomalyAttention TRN2 kernel: 8-core SPMD (2 batch-groups x 4 n-groups).

Module math (B=16, N=1024, C=D=256):
  Q,K,V = x@Wq, x@Wk, x@Wv ; sigma = x@Wsig
  P = rownorm(|i-j| + |sigma|*eps)
  S = softmax over BATCH dim of (Q K^T / 16)
  Z = S @ V
Returns (Z, P, S) like the reference.

Sharding: core c handles batches bh*8..bh*8+8 (bh=c%2) and rows
ng*256..ng*256+256 (ng=c//2). The batch-softmax denominator needs a
2-rank AllReduce between pair (2g, 2g+1).

Device computes scores in TRANSPOSED orientation (m on partitions) so
S^T feeds the Z matmul directly; the host transposes S back. Matmuls
run in float32r (tf32-like, full PE speed). K is never materialized:
scoresT = x @ (Wq Wk^T)^T_slice @ x_cols via Mt = Wq@Wk^T on device.
"""
import sys
sys.path.insert(0, '/opt/trn_rl_repo')
import numpy as np

B, N, C, D = 16, 1024, 256, 256
N_CORES = 8
B_LOC = 8      # batches per core
N_LOC = 256    # n rows per core
NT = N_LOC // 128   # 2 n-tiles
MT = N // 128       # 8 m-tiles
KC = C // 128       # 2 contraction tiles

_cached = {}

def _dma_chunks(nc, dst_fn, src_fn, total, chunk):
    for o in range(0, total, chunk):
        w = min(chunk, total - o)
        nc.sync.dma_start(dst_fn(o, w), src_fn(o, w))



def _build():
    import concourse.bass as bass
    import concourse.bacc as bacc
    import concourse.mybir as mybir
    from concourse import tile

    f32 = mybir.dt.float32
    f32r = mybir.dt.float32r
    AF = mybir.ActivationFunctionType
    ALU = mybir.AluOpType

    nc = bacc.Bacc("TRN2", target_bir_lowering=False, debug=False,
                   num_devices=N_CORES)

    xT = nc.dram_tensor("xT", [B_LOC, C, N], f32r, kind="ExternalInput")
    xq = nc.dram_tensor("xq", [B_LOC, C, N_LOC], f32r, kind="ExternalInput")
    eps = nc.dram_tensor("eps", [B_LOC, N_LOC, N], f32, kind="ExternalInput")
    p_in = nc.dram_tensor("p", [N_LOC, N], f32, kind="ExternalInput")
    wqT = nc.dram_tensor("wqT", [D, C], f32r, kind="ExternalInput")
    wkT = nc.dram_tensor("wkT", [D, C], f32r, kind="ExternalInput")
    wv = nc.dram_tensor("wv", [C, D], f32r, kind="ExternalInput")
    wsig = nc.dram_tensor("wsig", [C, 1], f32r, kind="ExternalInput")

    st_out = nc.dram_tensor("st_out", [B_LOC, N, N_LOC], f32,
                            kind="ExternalOutput")
    p_out = nc.dram_tensor("p_out", [B_LOC, N_LOC, N], f32,
                           kind="ExternalOutput")
    z_out = nc.dram_tensor("z_out", [B_LOC, N_LOC, D], f32,
                           kind="ExternalOutput")

    with tile.TileContext(nc) as tc:
        with tc.tile_pool(name="consts", bufs=1) as consts, \
             tc.tile_pool(name="xt", bufs=2) as xtp, \
             tc.tile_pool(name="xqp", bufs=2) as xqp, \
             tc.tile_pool(name="gp", bufs=2) as gp, \
             tc.tile_pool(name="big", bufs=1) as big, \
             tc.tile_pool(name="epsp", bufs=2) as epsp, \
             tc.tile_pool(name="dram", bufs=1, space="DRAM") as dram:

            # ---- constants ----
            wqT_sb = consts.tile([128, 2 * C], f32r)   # d-tile k at k*C
            wkT_sb = consts.tile([128, 2 * C], f32r)
            wv_sb = consts.tile([128, 2 * D], f32r)    # c-tile k at k*D
            wsig_sb = consts.tile([128, KC], f32r)
            p_sb = consts.tile([128, NT * N], f32)     # n-tile at nt*N
            mt_sb = consts.tile([128, 2 * C], f32r)    # Mt: c'-tile at k*C
            sig_sb = consts.tile([128, B_LOC * NT], f32)
            rs_sb = consts.tile([128, B_LOC * NT], f32)
            rinv_sb = consts.tile([128, B_LOC * NT], f32)

            for k in range(KC):
                nc.sync.dma_start(wqT_sb[:, k * C:(k + 1) * C],
                                  wqT.ap()[k * 128:(k + 1) * 128, :])
                nc.sync.dma_start(wkT_sb[:, k * C:(k + 1) * C],
                                  wkT.ap()[k * 128:(k + 1) * 128, :])
                nc.sync.dma_start(wv_sb[:, k * D:(k + 1) * D],
                                  wv.ap()[k * 128:(k + 1) * 128, :])
                nc.sync.dma_start(wsig_sb[:, k:k + 1],
                                  wsig.ap()[k * 128:(k + 1) * 128, :])
            for nt in range(NT):
                nc.sync.dma_start(p_sb[:, nt * N:(nt + 1) * N],
                                  p_in.ap()[nt * 128:(nt + 1) * 128, :])

            # ---- big persistent tiles ----
            expT = big.tile([128, MT * B_LOC * 256], f32)  # 8MB: mt-major, b at b*256
            d_loc = big.tile([128, MT * 256], f32)         # 1MB; reused as R after AR
            v_all = big.tile([128, B_LOC * MT * 256], f32r)  # 8MB: (b*8+mt)*256

            ar_in = dram.tile([128, MT * 256], f32)
            ar_out = dram.tile([128, MT * 256], f32)

            with tc.tile_pool(name="psab", bufs=6, space="PSUM") as psab:
                # Mt = Wq @ Wk^T  (lhsT = wqT slices, rhs = wkT)
                for cpt in range(KC):
                    ps = psab.tile([128, 512], f32, tag="ps")
                    for kd in range(KC):
                        nc.tensor.matmul(
                            ps[:, 0:C],
                            wqT_sb[:, kd * C + cpt * 128: kd * C + cpt * 128 + 128],
                            wkT_sb[:, kd * C:(kd + 1) * C],
                            start=(kd == 0), stop=(kd == KC - 1))
                    nc.vector.tensor_copy(mt_sb[:, cpt * C:(cpt + 1) * C],
                                          ps[:, 0:C])

                for b in range(B_LOC):
                    # ---- loads ----
                    xt_t = xtp.tile([128, KC * N], f32r, tag="xt")
                    for k in range(KC):
                        for o in range(0, N, 512):
                            nc.sync.dma_start(
                                xt_t[:, k * N + o:k * N + o + 512],
                                xT.ap()[b, k * 128:(k + 1) * 128, o:o + 512])
                    xq_t = xqp.tile([128, KC * N_LOC], f32r, tag="xq")
                    for k in range(KC):
                        nc.sync.dma_start(xq_t[:, k * N_LOC:(k + 1) * N_LOC],
                                          xq.ap()[b, k * 128:(k + 1) * 128, :])

                    # ---- G[b] = Mt^T-style: out [c-tile, n] ----
                    g_t = gp.tile([128, KC * N_LOC], f32r, tag="g")
                    for ct in range(KC):
                        ps = psab.tile([128, 512], f32, tag="ps")
                        for kcp in range(KC):
                            nc.tensor.matmul(
                                ps[:, 0:N_LOC],
                                mt_sb[:, kcp * C + ct * 128: kcp * C + ct * 128 + 128],
                                xq_t[:, kcp * N_LOC:(kcp + 1) * N_LOC],
                                start=(kcp == 0), stop=(kcp == KC - 1))
                        nc.vector.tensor_copy(
                            g_t[:, ct * N_LOC:(ct + 1) * N_LOC],
                            ps[:, 0:N_LOC])

                    # ---- sigma[b] ----
                    for nt in range(NT):
                        ps = psab.tile([128, 512], f32, tag="ps")
                        for k in range(KC):
                            nc.tensor.matmul(
                                ps[:, 0:1],
                                xq_t[:, k * N_LOC + nt * 128:
                                     k * N_LOC + nt * 128 + 128].bitcast(f32),
                                wsig_sb[:, k:k + 1].bitcast(f32),
                                start=(k == 0), stop=(k == KC - 1))
                        slot = b * NT + nt
                        nc.scalar.activation(sig_sb[:, slot:slot + 1],
                                             ps[:, 0:1], AF.Abs)

                    # ---- scoresT + exp (mt pairs share a psum bank) ----
                    for mtp in range(MT // 2):
                        ps = psab.tile([128, 512], f32, tag="ps")
                        for sub in range(2):
                            mt = 2 * mtp + sub
                            for k in range(KC):
                                nc.tensor.matmul(
                                    ps[:, sub * 256:sub * 256 + N_LOC],
                                    xt_t[:, k * N + mt * 128:
                                         k * N + mt * 128 + 128],
                                    g_t[:, k * N_LOC:(k + 1) * N_LOC],
                                    start=(sub == 0 and k == 0),
                                    stop=(sub == 1 and k == KC - 1),
                                    skip_group_check=True)
                        out_ap = (expT.rearrange("p (m b n) -> p m b n",
                                                  m=MT, b=B_LOC)
                                  [:, 2 * mtp:2 * mtp + 2, b, :])
                        nc.scalar.activation(out_ap, ps[:]
                                             .rearrange("p (m n) -> p m n",
                                                        m=2),
                                             AF.Exp, scale=1.0 / 16.0)

                    # ---- V[b] (mt pairs share a psum bank) ----
                    for mtp in range(MT // 2):
                        ps = psab.tile([128, 512], f32, tag="ps")
                        for sub in range(2):
                            mt = 2 * mtp + sub
                            for k in range(KC):
                                nc.tensor.matmul(
                                    ps[:, sub * 256:sub * 256 + D],
                                    xt_t[:, k * N + mt * 128:
                                         k * N + mt * 128 + 128],
                                    wv_sb[:, k * D:(k + 1) * D],
                                    start=(sub == 0 and k == 0),
                                    stop=(sub == 1 and k == KC - 1),
                                    skip_group_check=True)
                        voff = (b * MT + 2 * mtp) * 256
                        nc.vector.tensor_copy(v_all[:, voff:voff + 512],
                                              ps[:])

                    # ---- incremental D sums after each odd b ----
                    if b % 2 == 1:
                        for mt in range(MT):
                            base = mt * B_LOC * 256
                            e0 = expT[:, base + (b - 1) * 256: base + b * 256]
                            e1 = expT[:, base + b * 256: base + (b + 1) * 256]
                            dsl = d_loc[:, mt * 256:(mt + 1) * 256]
                            if b == 1:
                                nc.vector.tensor_add(dsl, e0, e1)
                            else:
                                nc.vector.tensor_add(dsl, dsl, e0)
                                nc.vector.tensor_add(dsl, dsl, e1)

                # ---- AllReduce of d_loc between pair ----
                for o in range(0, MT * 256, 512):
                    nc.sync.dma_start(ar_in[:, o:o + 512],
                                      d_loc[:, o:o + 512])
                nc.gpsimd.collective_compute(
                    "AllReduce", ALU.add,
                    replica_groups=[[0, 1], [2, 3], [4, 5], [6, 7]],
                    ins=[ar_in.opt()], outs=[ar_out.opt()])
                for o in range(0, MT * 256, 512):
                    nc.sync.dma_start(d_loc[:, o:o + 512],
                                      ar_out[:, o:o + 512])
                nc.vector.reciprocal_approx_fast(d_loc[:], d_loc[:])

                # ---- P phase (fills the AllReduce window) ----
                for b in range(B_LOC):
                    for nt in range(NT):
                        slot = b * NT + nt
                        ep = epsp.tile([128, N], f32, tag="eps")
                        for o in range(0, N, 512):
                            nc.sync.dma_start(
                                ep[:, o:o + 512],
                                eps.ap()[b, nt * 128:(nt + 1) * 128,
                                         o:o + 512])
                        nc.vector.scalar_tensor_tensor(
                            ep[:], ep[:], sig_sb[:, slot:slot + 1],
                            p_sb[:, nt * N:(nt + 1) * N],
                            ALU.mult, ALU.add,
                            accum_out=rs_sb[:, slot:slot + 1])
                        nc.vector.reciprocal(rinv_sb[:, slot:slot + 1],
                                             rs_sb[:, slot:slot + 1])
                        nc.scalar.activation(ep[:], ep[:], AF.Copy,
                                             scale=rinv_sb[:, slot:slot + 1])
                        for o in range(0, N, 512):
                            nc.sync.dma_start(
                                p_out.ap()[b, nt * 128:(nt + 1) * 128,
                                           o:o + 512],
                                ep[:, o:o + 512])

            # ---- phase D: S^T, S out, Z ----
            with tc.tile_pool(name="psz", bufs=1, space="PSUM") as psz, \
                 tc.tile_pool(name="zsbp", bufs=2) as zsbp, \
                 tc.tile_pool(name="stp", bufs=2) as stp:
                zps = [psz.tile([128, 512], f32, tag=f"z{i}",
                                name=f"zp{i}") for i in range(B_LOC)]
                for mt in range(MT):
                    sl = expT[:, mt * B_LOC * 256:(mt + 1) * B_LOC * 256]
                    src = sl.rearrange("p (b n) -> p b n", b=B_LOC)
                    st_t = stp.tile([128, B_LOC * 256], f32r, tag="st")
                    dst = st_t.rearrange("p (b n) -> p b n", b=B_LOC)
                    rb = (d_loc[:, mt * 256:(mt + 1) * 256]
                          .rearrange("p (o n) -> p o n", o=1)
                          .broadcast_to([128, B_LOC, 256]))
                    nc.vector.tensor_mul(dst, src, rb)
                    # write S^T slab: [128(m), b, n] -> st_out[b, m, n]
                    for bo in range(0, B_LOC, 2):
                        nc.sync.dma_start(
                            st_out.ap()[bo:bo + 2, mt * 128:(mt + 1) * 128, :]
                            .rearrange("b m n -> m b n"),
                            st_t[:, bo * 256:(bo + 2) * 256].bitcast(f32))
                    # Z accumulation
                    for b in range(B_LOC):
                        for nt in range(NT):
                            lhs = st_t[:, b * 256 + nt * 128:
                                       b * 256 + nt * 128 + 128]
                            voff = (b * MT + mt) * 256
                            nc.tensor.matmul(
                                zps[b][:, nt * 256:(nt + 1) * 256],
                                lhs, v_all[:, voff:voff + 256],
                                start=(mt == 0 and nt == 0),
                                stop=(mt == MT - 1 and nt == NT - 1),
                                skip_group_check=True)
                for b in range(B_LOC):
                    zsb = zsbp.tile([128, 512], f32, tag="zsb")
                    nc.scalar.activation(zsb[:], zps[b][:], AF.Copy)
                    for nt in range(NT):
                        nc.sync.dma_start(
                            z_out.ap()[b, nt * 128:(nt + 1) * 128, :],
                            zsb[:, nt * 256:(nt + 1) * 256])
    nc.compile()
    return nc


def _get_nc():
    if "nc" not in _cached:
        _cached["nc"] = _build()
    return _cached["nc"]


def kernel(x, W_q, W_k, W_v, W_sigma, eps):
    from concourse.bass_utils import run_bass_kernel_spmd

    x = np.ascontiguousarray(x, dtype=np.float32)
    eps = np.ascontiguousarray(eps, dtype=np.float32)
    W_q = np.ascontiguousarray(W_q, dtype=np.float32)
    W_k = np.ascontiguousarray(W_k, dtype=np.float32)
    W_v = np.ascontiguousarray(W_v, dtype=np.float32)
    W_sigma = np.ascontiguousarray(W_sigma, dtype=np.float32)

    xT_f = np.ascontiguousarray(x.transpose(0, 2, 1))  # [B, C, N]
    idx = np.arange(N, dtype=np.float32)
    p_full = np.abs(idx[:, None] - idx[None, :])       # [N, N]
    wqT = np.ascontiguousarray(W_q.T)
    wkT = np.ascontiguousarray(W_k.T)

    in_maps = []
    for c in range(N_CORES):
        bh, ng = c % 2, c // 2
        bs, ns = bh * B_LOC, ng * N_LOC
        in_maps.append({
            "xT": xT_f[bs:bs + B_LOC],
            "xq": np.ascontiguousarray(xT_f[bs:bs + B_LOC, :, ns:ns + N_LOC]),
            "eps": np.ascontiguousarray(eps[bs:bs + B_LOC, ns:ns + N_LOC, :]),
            "p": np.ascontiguousarray(p_full[ns:ns + N_LOC, :]),
            "wqT": wqT, "wkT": wkT, "wv": W_v, "wsig": W_sigma,
        })

    nc = _get_nc()
    res = run_bass_kernel_spmd(nc, in_maps, core_ids=list(range(N_CORES)))
    _cached["last_res"] = res

    Z = np.empty((B, N, D), dtype=np.float32)
    P = np.empty((B, N, N), dtype=np.float32)
    S = np.empty((B, N, N), dtype=np.float32)
    for c in range(N_CORES):
        bh, ng = c % 2, c // 2
        bs, ns = bh * B_LOC, ng * N_LOC
        r = res.results[c]
        Z[bs:bs + B_LOC, ns:ns + N_LOC, :] = r["z_out"]
        P[bs:bs + B_LOC, ns:ns + N_LOC, :] = r["p_out"]
        S[bs:bs + B_LOC, ns:ns + N_LOC, :] = r["st_out"].transpose(0, 2, 1)
    return Z, P, S
